# revision 22
# baseline (speedup 1.0000x reference)
"""GQA attention (RoPE + causal softmax + out-proj) on 8 Trainium2 cores, v2.

Sharding: DP=2 over batch x TP=4 over KV groups (core c: batch c//4, tp c%4).
All matmuls in bf16 (PE 1 cyc/row, FWL weight loads), PSUM accumulation fp32.

Per-core pipeline (single Tile program, no DRAM round-trips for q/k/v):
  1. QKV projection from host-pretransposed xT (bf16) streamed in 512-token
     chunks; RoPE on DVE; q^T/k^T/v kept SBUF-resident.
  2. Flash-style causal attention per head in S^T orientation; exp on ACT
     (bf16 out), denominator via DVE bf16 adds + ones-matmul; PV in PSUM.
  3. Out-projection split into 4 passes over head groups {0-3},{4,5},{6},{7}.
     Each group is AllGathered (bf16) as soon as its heads finish, and its
     partial y accumulates in SBUF while later heads still run attention —
     hiding both the collective and most of the out-proj behind attention.
"""
import sys

sys.path.insert(0, "/opt/trn_rl_repo")

import numpy as np
import ml_dtypes

import concourse.bacc as bacc
import concourse.mybir as mybir
from concourse import tile
from concourse.bass_utils import run_bass_kernel_spmd

F32 = mybir.dt.float32
BF16 = mybir.dt.bfloat16
EXP = mybir.ActivationFunctionType.Exp
MULT = mybir.AluOpType.mult
ADD = mybir.AluOpType.add

BF = ml_dtypes.bfloat16

# out-proj passes: (head_lo, head_hi) per pass
PASSES = [(0, 4), (4, 6), (6, 7), (7, 8)]


class Cfg:
    def __init__(self, B=2, S=2048, D=4096, H=32, KV=8, HD=128, TP=4,
                 TCHUNK=1024, NQ=512):
        self.B, self.S, self.D, self.H, self.KV, self.HD = B, S, D, H, KV, HD
        self.TP = TP
        self.DP = B
        self.NCORES = TP * B
        self.G = H // KV                    # q heads per kv head
        self.HL = H // TP                   # local q heads (8)
        self.KVL = KV // TP                 # local kv heads (2)
        self.DT = D // 128                  # contraction d-tiles (32)
        self.QCOLS = self.HL * HD           # 1024
        self.KCOLS = self.KVL * HD          # 256
        self.WCOLS = self.QCOLS + 2 * self.KCOLS  # 1536
        self.TCHUNK = TCHUNK
        self.NCH = S // TCHUNK              # 4
        self.NQ = NQ                        # attention qt-chunk
        self.NMASK = NQ // 128
        self.NQC = S // NQ
        self.OUTC = self.D // TP            # local wo out-cols (1024)
        self.AT = (H * HD) // 128           # total attnout hd-tiles (32)
        self.TH = TCHUNK // 512             # 512-token halves per chunk
        assert HD == 128 and TCHUNK % 512 == 0 and NQ % 128 == 0


def _make_sets(cfg):
    """Stage-1 column sets: uniform [128, 512tok] w-stationary psum groups.
    Group: (kind, idx, th, lo, hi); kind "q"|"k"|"vt" (vt = transposed v,
    [128 vcol, 512 tok], PE-transposed to token-major after evacuation).
    th-pairs are kept adjacent so consecutive matmuls reuse the stationary
    weight tile (HW skips the redundant PE weight reload)."""
    groups = []
    for h in range(cfg.HL):
        for th in range(cfg.TH):
            groups.append(("q", h, th, h * 128, (h + 1) * 128))
    for kh in range(cfg.KVL):
        for th in range(cfg.TH):
            groups.append(("k", kh, th, cfg.QCOLS + kh * 128,
                           cfg.QCOLS + (kh + 1) * 128))
    vlo = cfg.QCOLS + cfg.KCOLS
    for vc in range(cfg.KCOLS // 128):
        for th in range(cfg.TH):
            groups.append(("vt", vc, th, vlo + vc * 128,
                           vlo + (vc + 1) * 128))
    sizes = [5, 5, 5, 5, 4]
    assert sum(sizes) == len(groups)
    sets, i = [], 0
    for sz in sizes:
        grps = groups[i:i + sz]
        i += sz
        lo = min(g[3] for g in grps)
        hi = max(g[4] for g in grps)
        sets.append((lo, hi, grps))
    return sets


def build_program(cfg, num_devices, replica_groups, fake_ag=False,
                  dummy_io=False, repeat=1):
    nc = bacc.Bacc(trn_type="TRN2", target_bir_lowering=False, debug=False,
                   num_devices=num_devices)
    S, D, HD = cfg.S, cfg.D, cfg.HD
    scale = float(1.0 / np.sqrt(HD))
    NQ, NQC = cfg.NQ, cfg.NQC

    o1_d = nc.dram_tensor("ones1", [128, 1], BF16, kind="ExternalInput").ap()
    o2_d = nc.dram_tensor("ones2", [1, 128], BF16, kind="ExternalInput").ap()
    id_d = nc.dram_tensor("ident", [128, 128], BF16,
                          kind="ExternalInput").ap()
    if dummy_io:
        y_d = nc.dram_tensor("ydum", [128, 4], BF16, kind="ExternalOutput").ap()
    else:
        xT_d = nc.dram_tensor("xT", [D, S], BF16, kind="ExternalInput").ap()
        w_d = nc.dram_tensor("wqkvT", [D, cfg.WCOLS], BF16,
                             kind="ExternalInput").ap()
        wop_d = [nc.dram_tensor(f"woT{p}", [(h1 - h0) * cfg.TP * 128,
                                            cfg.OUTC], BF16,
                                kind="ExternalInput").ap()
                 for p, (h0, h1) in enumerate(PASSES)]
        p1_d = nc.dram_tensor("p1", [128, S], BF16, kind="ExternalInput").ap()
        p2_d = nc.dram_tensor("p2", [128, S], BF16, kind="ExternalInput").ap()
        mk_d = nc.dram_tensor("masks", [128, cfg.NMASK, cfg.NQ], BF16,
                              kind="ExternalInput").ap()
        y_d = nc.dram_tensor("y", [S, cfg.OUTC], F32,
                             kind="ExternalOutput").ap()

    sets = _make_sets(cfg)
    wmax = max(hi - lo for (lo, hi, _) in sets)

    with tile.TileContext(nc) as tc, nc.allow_low_precision(
            reason="bf16 compute; fp32 PSUM accumulation"):
        with (
            tc.tile_pool(name="dram", bufs=1, space="DRAM") as pd,
            tc.tile_pool(name="const", bufs=1) as pc,
        ):
            if dummy_io:
                xT_d = pd.tile([D, S], BF16)
                w_d = pd.tile([D, cfg.WCOLS], BF16)
                wop_d = [pd.tile([(h1 - h0) * cfg.TP * 128, cfg.OUTC], BF16,
                                 name=f"wopd{p}")
                         for p, (h0, h1) in enumerate(PASSES)]
                p1_d = pd.tile([128, S], BF16)
                p2_d = pd.tile([128, S], BF16)
                mk_d = pd.tile([128, cfg.NMASK, cfg.NQ], BF16)
                yr_d = pd.tile([S, cfg.OUTC], F32)
            ao_d = [pd.tile([(h1 - h0) * 128, S], BF16, name=f"aod{p}")
                    for p, (h0, h1) in enumerate(PASSES)]
            ag_d = [pd.tile([(h1 - h0) * cfg.TP * 128, S], BF16,
                            name=f"agd{p}")
                    for p, (h0, h1) in enumerate(PASSES)]

            ones = pc.tile([128, 1], BF16)
            ones_r = pc.tile([1, 128], BF16)
            ident = pc.tile([128, 128], BF16)
            nc.gpsimd.dma_start(ones[:], o1_d)
            nc.gpsimd.dma_start(ones_r[:], o2_d)
            nc.gpsimd.dma_start(ident[:], id_d)

            import contextlib
            loop_cm = (tc.For_i(0, repeat, 1, hint_engines=(
                           mybir.EngineType.PE, mybir.EngineType.DVE,
                           mybir.EngineType.Activation)) if repeat > 1
                       else contextlib.nullcontext())
            loop_cm.__enter__()
            with (
                tc.tile_pool(name="qkv", bufs=1) as pqkv,
                tc.tile_pool(name="s23c", bufs=1) as pc2,
            ):
                qT_sb = pqkv.tile([128, cfg.HL, S], BF16)
                kT_sb = pqkv.tile([128, cfg.KVL, S], BF16)
                v_sb = pqkv.tile([128, S // 128, cfg.KCOLS], BF16)

                # ---------------- stage 1: QKV + RoPE ----------------
                with (
                    tc.tile_pool(name="s1c", bufs=1) as pc1,
                    tc.tile_pool(name="s1xT", bufs=2) as pxT,
                    tc.tile_pool(name="s1w", bufs=3) as pw,
                    tc.tile_pool(name="s1t", bufs=4) as pt,
                    tc.tile_pool(name="s1ps", bufs=6, space="PSUM") as pps,
                    tc.tile_pool(name="s1tr", bufs=2, space="PSUM") as ptr,
                ):
                    p1s = pc1.tile([128, S], BF16)
                    p2s = pc1.tile([128, S], BF16)
                    p_loaded = False
                    prev_w2 = None     # prior chunk's set-2 first w load
                    for c0 in range(cfg.NCH):
                        w2_first = None
                        t0 = c0 * cfg.TCHUNK
                        # xT in two half-tiles (dt 0-15 / 16-31) so chunk
                        # c+1's first half can load while chunk c finishes
                        xTh = [pxT.tile([128, cfg.DT // 2, cfg.TCHUNK],
                                        BF16, tag="xT", bufs=3, name="xTh")
                               for _ in range(2)]

                        def xts(dt):
                            return xTh[dt // 16][:, dt % 16, :]
                        si = 0
                        for (lo, hi, grps) in sets:
                            si += 1
                            wid = hi - lo
                            psums = [pps.tile([128, 512], F32, tag="ps",
                                              name="ps") for _ in grps]
                            for dtb in range(cfg.DT // 4):
                                wt = pw.tile([128, 4, wmax], BF16,
                                             tag=f"wt{si % 2}", bufs=4,
                                             name="wt")
                                dma_w = nc.sync if (dtb + si) % 2 else nc.scalar
                                wbi = dma_w.dma_start(
                                    wt[:, :, 0:wid],
                                    w_d[dtb * 512:(dtb + 1) * 512, lo:hi]
                                    .rearrange("(f p) c -> p f c", p=128))
                                if si == 2 and dtb == 0:
                                    w2_first = wbi.ins
                                if si == 1:
                                    # stream this dtb's xT d-tiles right
                                    # behind its weights
                                    d0 = dtb * 4
                                    hf = d0 // 16
                                    o0 = d0 % 16
                                    dma_x = (nc.scalar if (dtb + si) % 2
                                             else nc.sync)
                                    xbi = dma_x.dma_start(
                                        xTh[hf][:, o0:o0 + 4, :],
                                        xT_d[d0 * 128:(d0 + 4) * 128,
                                             t0:t0 + cfg.TCHUNK]
                                        .rearrange("(dt p) t -> p dt t",
                                                   p=128))
                                    if prev_w2 is not None:
                                        tile.add_dep_helper(
                                            xbi.ins, prev_w2, sync=False,
                                            reason="xT prefetch yields")
                                for ds in range(4):
                                    dt = dtb * 4 + ds
                                    st = (dt == 0)
                                    sp = (dt == cfg.DT - 1)
                                    for ps, (kind, a, th, glo,
                                             ghi) in zip(psums, grps):
                                        nc.tensor.matmul(
                                            ps[:],
                                            wt[:, ds, glo - lo:ghi - lo],
                                            xts(dt)[:,
                                                    th * 512:
                                                    th * 512 + 512],
                                            start=st, stop=sp)
                            if not p_loaded:
                                nc.sync.dma_start(p1s[:], p1_d)
                                nc.scalar.dma_start(p2s[:], p2_d)
                                p_loaded = True
                            if si == 3:
                                prev_w2 = w2_first
                            # evacuate psums
                            for ps, (kind, a, th, glo, ghi) in zip(psums,
                                                                   grps):
                                cl0 = t0 + th * 512
                                if kind == "vt":
                                    # vT [128 vcol, 512 tok] -> staging, then
                                    # PE-transpose 128x128 blocks into v_sb
                                    # (token-major, as PV's stationary).
                                    vstg = pt.tile([128, 512], BF16,
                                                   tag="vstg", bufs=2,
                                                   name="vstg")
                                    nc.vector.tensor_copy(vstg[:], ps[:])
                                    for j in range(4):
                                        tp = ptr.tile([128, 128], BF16,
                                                      tag="tp", name="tp")
                                        nc.tensor.matmul(
                                            tp[:],
                                            vstg[:, j * 128:(j + 1) * 128],
                                            ident[:],
                                            start=True, stop=True,
                                            is_transpose=True)
                                        nc.vector.tensor_copy(
                                            v_sb[:, cl0 // 128 + j,
                                                 a * 128:(a + 1) * 128],
                                            tp[:])
                                else:
                                    # RoPE: out = in*P1 + swap(in)*P2.
                                    # Swap muls must read PSUM (cross-base-
                                    # partition SBUF+SBUF is illegal); the
                                    # aligned mul uses a bf16 ACT copy for
                                    # the DVE 2-byte fast path.
                                    cl = t0 + th * 512
                                    ch = cl + 512
                                    pb = pt.tile([128, 512], BF16, tag="pb")
                                    nc.scalar.copy(pb[:], ps[:])
                                    t1 = pt.tile([128, 512], BF16, tag="t1")
                                    t2 = pt.tile([128, 512], BF16, tag="t2")
                                    nc.vector.tensor_mul(
                                        t1[:], pb[:], p1s[:, cl:ch])
                                    nc.vector.tensor_mul(
                                        t2[0:64, :], ps[64:128, :],
                                        p2s[0:64, cl:ch])
                                    nc.vector.tensor_mul(
                                        t2[64:128, :], ps[0:64, :],
                                        p2s[64:128, cl:ch])
                                    dst = (qT_sb[:, a, cl:ch] if kind == "q"
                                           else kT_sb[:, a, cl:ch])
                                    nc.vector.tensor_add(dst, t1[:], t2[:])

                # ---------------- stage 2+3: attention + out-proj ----------
                with (
                    tc.tile_pool(name="wo", bufs=1) as pwo,
                    tc.tile_pool(name="ys", bufs=1) as pys,
                    tc.tile_pool(name="s2w", bufs=4) as pw2,
                    tc.tile_pool(name="s2a", bufs=2) as pao,
                    tc.tile_pool(name="s3a", bufs=2) as pa3,
                    tc.tile_pool(name="s3y", bufs=2) as py3,
                    tc.tile_pool(name="s2ps", bufs=2, space="PSUM") as pps2,
                    tc.tile_pool(name="s2pd", bufs=1, space="PSUM") as ppsd,
                    tc.tile_pool(name="s3ps", bufs=2, space="PSUM") as pps3,
                ):
                    ys = pys.tile([128, S // 128, cfg.OUTC], BF16)
                    msk = pc2.tile([128, cfg.NMASK, cfg.NQ], BF16)
                    nc.sync.dma_start(msk[:], mk_d)
                    wo_sb = {}
                    ao_writes = {p: [] for p in range(len(PASSES))}
                    ag_insts = {p: [] for p in range(len(PASSES))}

                    def load_wo(p, part=None, nparts=1):
                        h0, h1 = PASSES[p]
                        na = (h1 - h0) * cfg.TP
                        if p not in wo_sb:
                            wo_sb[p] = pwo.tile([128, na, cfg.OUTC], BF16,
                                                tag=f"wo{p}", name=f"wo{p}")
                        a0 = 0 if part is None else part * (na // nparts)
                        a1 = na if part is None else a0 + na // nparts
                        dma = nc.sync if (a0 // 4) % 2 else nc.scalar
                        dma.dma_start(
                            wo_sb[p][:, a0:a1, :],
                            wop_d[p][a0 * 128:a1 * 128, :]
                            .rearrange("(a p) o -> p a o", p=128))

                    def emit_ag(p, h):
                        """AllGather ONE head's attnout across the TP group.
                        ag_d[p] rows are head-major: [(h-h0)*512 + r*128]."""
                        h0, h1 = PASSES[p]
                        hh = h - h0
                        src = ao_d[p][hh * 128:(hh + 1) * 128, :]
                        new = []
                        if fake_ag:
                            for r in range(cfg.TP):
                                ro = hh * 512 + r * 128
                                bi = nc.gpsimd.dma_start(
                                    ag_d[p][ro:ro + 128, :], src)
                                new.append(bi.ins)
                        else:
                            bi = nc.gpsimd.collective_compute(
                                "AllGather", mybir.AluOpType.bypass,
                                replica_groups=replica_groups,
                                ins=[src.opt()],
                                outs=[ag_d[p][hh * 512:(hh + 1) * 512, :]
                                      .opt()])
                            new.append(bi.ins)
                        for gi in new:
                            tile.add_dep_helper(gi, ao_writes[p][-1],
                                                reason="ao->AG RAW")
                        ag_insts[p].extend(new)

                    def emit_pass(p):
                        h0, h1 = PASSES[p]
                        na = (h1 - h0) * cfg.TP
                        # token-tiles per at load: 2 for wide passes, 4 for
                        # narrow (na<=4) passes so the tail has half the
                        # loads (1KB descriptor rows either way)
                        ntl = 4 if na <= 4 else 2
                        for tpn in range(S // (128 * ntl)):
                            at = pa3.tile([128, 16 // (ntl // 2),
                                           128 * ntl], BF16, tag="at",
                                          bufs=2, name="at")
                            bi = nc.sync.dma_start(
                                at[:, 0:na, :],
                                ag_d[p][:, tpn * 128 * ntl:
                                        (tpn + 1) * 128 * ntl]
                                .rearrange("(a p) t -> p a t", p=128))
                            for gi in ag_insts[p]:
                                tile.add_dep_helper(bi.ins, gi,
                                                    reason="AG->at RAW")
                            for ti in range(ntl):
                                tt = tpn * ntl + ti
                                yf = None
                                if p == 3:
                                    yf = py3.tile([128, cfg.OUTC], F32,
                                                  tag="yf", name="yf")
                                # oc-inner so consecutive matmuls reuse the
                                # stationary at[:, a] tile (skips the HW
                                # weight reload on every other matmul)
                                ypss = [pps3.tile([128, 512], F32,
                                                  tag="yps", bufs=3,
                                                  name="yps")
                                        for oc in range(2)]
                                for a in range(na):
                                    for oc in range(2):
                                        nc.tensor.matmul(
                                            ypss[oc][:],
                                            at[:, a,
                                               ti * 128:(ti + 1) * 128],
                                            wo_sb[p][:, a,
                                                     oc * 512:
                                                     (oc + 1) * 512],
                                            start=(a == 0),
                                            stop=(a == na - 1))
                                for oc in range(2):
                                    yps = ypss[oc]
                                    ysl = ys[:, tt, oc * 512:(oc + 1) * 512]
                                    if p == 0:
                                        nc.vector.tensor_copy(ysl, yps[:])
                                    elif p < 3:
                                        nc.vector.tensor_add(ysl, ysl,
                                                             yps[:])
                                    else:
                                        nc.vector.tensor_add(
                                            yf[:, oc * 512:(oc + 1) * 512],
                                            ysl, yps[:])
                                        if oc == 1:
                                            nc.scalar.dma_start(
                                                (yr_d if dummy_io else y_d)
                                                [tt * 128:(tt + 1) * 128,
                                                 :],
                                                yf[:])

                    pending_tail = None
                    pending_b = None
                    for h in range(cfg.HL):
                        kv = h // cfg.G
                        if h < 4:
                            load_wo(0, part=h, nparts=4)
                        aost = pao.tile([128, S], BF16, tag="aost")
                        for qc in range(NQC):
                            nkt = (qc + 1) * (NQ // 128)
                            cl = qc * NQ
                            ch = cl + NQ
                            acc_e = pw2.tile([128, NQ], BF16, tag="acce",
                                             bufs=2)
                            acc_o = pw2.tile([128, NQ], BF16, tag="acco",
                                             bufs=2)
                            aop = pps2.tile([128, NQ], F32, tag="aop",
                                            bufs=2)
                            for ki in range(nkt):
                                if ki == 2 and pending_tail is not None:
                                    pending_b = pending_tail()
                                    pending_tail = None
                                if ki == 3 and pending_b is not None:
                                    pending_b()
                                    pending_b = None
                                di = ki - qc * (NQ // 128)
                                # Diagonal tiles: columns < 128*di are fully
                                # masked — skip them in scores/exp/PV when
                                # the accumulators are already initialized.
                                j0 = 128 * di if (di > 0 and ki >= 2) else 0
                                sps = pps2.tile([128, NQ], F32, tag="sps",
                                                bufs=3)
                                nc.tensor.matmul(
                                    sps[:, j0:NQ],
                                    kT_sb[:, kv, ki * 128:(ki + 1) * 128],
                                    qT_sb[:, h, cl + j0:ch],
                                    start=True, stop=True)
                                et = pw2.tile([128, NQ], BF16, tag="et",
                                              bufs=4)
                                nc.scalar.activation(et[:, j0:NQ],
                                                     sps[:, j0:NQ], EXP,
                                                     scale=scale)
                                if di >= 0:
                                    mt = pw2.tile([128, NQ], BF16, tag="mt",
                                                  bufs=4)
                                    if j0 > 0:
                                        nc.vector.tensor_mul(
                                            mt[:, j0:NQ], et[:, j0:NQ],
                                            msk[:, 0, 0:NQ - j0])
                                    else:
                                        nc.vector.tensor_mul(mt[:], et[:],
                                                             msk[:, di, :])
                                    use = mt
                                else:
                                    use = et
                                nc.tensor.matmul(
                                    aop[:, j0:NQ],
                                    v_sb[:, ki, kv * 128:(kv + 1) * 128],
                                    use[:, j0:NQ],
                                    start=(ki == 0), stop=(ki == nkt - 1))
                                dst = acc_e if ki % 2 == 0 else acc_o
                                if ki < 2:
                                    nc.vector.tensor_copy(dst[:], use[:])
                                else:
                                    nc.vector.tensor_add(dst[:, j0:NQ],
                                                         dst[:, j0:NQ],
                                                         use[:, j0:NQ])
                            # Denominator tail: deferred into the next qc's
                            # ki stream so the PE never idles on the DVE
                            # accr->recip chain; dspt/rbpt reuse the sps psum
                            # tag rotation (no extra PSUM banks).
                            def make_tail(h=h, qc=qc, cl=cl, ch=ch,
                                          acc_e=acc_e, acc_o=acc_o, aop=aop,
                                          aost=aost, last=(qc == NQC - 1)):
                                accr = pw2.tile([128, NQ], BF16, tag="accr",
                                                bufs=2, name="accr")
                                nc.vector.tensor_tensor(accr[:], acc_e[:],
                                                        acc_o[:], op=ADD)
                                dspt = pps2.tile([128, NQ], F32, tag="sps",
                                                 bufs=3, name="dspt")
                                nc.tensor.matmul(dspt[0:1, :], ones[:],
                                                 accr[:],
                                                 start=True, stop=True)
                                rec = pw2.tile([1, NQ], BF16, tag="rec",
                                               bufs=2, name="rec")
                                nc.vector.reciprocal(rec[:], dspt[0:1, :])

                                def part_b():
                                    rbpt = pps2.tile([128, NQ], F32,
                                                     tag="sps", bufs=3,
                                                     name="rbpt")
                                    nc.tensor.matmul(rbpt[:], ones_r[:],
                                                     rec[:],
                                                     start=True, stop=True)
                                    rbs = pw2.tile([128, NQ], F32,
                                                   tag="rbs", bufs=2,
                                                   name="rbs")
                                    nc.vector.tensor_copy(rbs[:], rbpt[:])
                                    nc.vector.tensor_tensor(
                                        aost[:, cl:ch], aop[:], rbs[:],
                                        op=MULT)
                                    if last:
                                        # ship head to its pass's ao buffer
                                        for p, (h0, h1) in enumerate(PASSES):
                                            if h0 <= h < h1:
                                                bi = nc.scalar.dma_start(
                                                    ao_d[p][(h - h0) * 128:
                                                            (h - h0 + 1)
                                                            * 128, :],
                                                    aost[:])
                                                ao_writes[p].append(bi.ins)
                                                emit_ag(p, h)
                                return part_b

                            pending_tail = make_tail
                        if h == 2:
                            load_wo(1, part=0, nparts=2)
                        if h == 3:
                            load_wo(1, part=1, nparts=2)
                        if h == 4:
                            load_wo(2)
                        if h == 5:
                            emit_pass(0)
                        if h == 5:
                            load_wo(3)
                        if h == 6:
                            emit_pass(1)
                    if pending_tail is not None:
                        pending_b = pending_tail()
                        pending_tail = None
                    if pending_b is not None:
                        pending_b()
                        pending_b = None
                    emit_pass(2)
                    emit_pass(3)
            loop_cm.__exit__(None, None, None)
            if dummy_io:
                nc.sync.dma_start(y_d[:, 0:1], ones[:])

    nc.compile()
    return nc


def host_prep(cfg, x, freq_cis, wq, wk, wv, wo):
    """Per-core input maps: shard + transpose + bf16 cast on host."""
    HD, S = cfg.HD, cfg.S
    perm = np.concatenate([np.arange(0, HD, 2), np.arange(1, HD, 2)])
    fc = np.asarray(freq_cis, np.float32)
    A = fc[:, :, 0, 0].T
    Bc = fc[:, :, 0, 1].T
    C = fc[:, :, 1, 0].T
    Dd = fc[:, :, 1, 1].T
    p1 = np.ascontiguousarray(np.concatenate([A, Dd], 0)).astype(BF)
    p2 = np.ascontiguousarray(np.concatenate([Bc, C], 0)).astype(BF)
    i_idx = np.arange(128)[:, None]
    j_idx = np.arange(cfg.NQ)[None, :]
    masks = np.stack([(j_idx >= i_idx + 128 * di).astype(np.float32)
                      for di in range(cfg.NMASK)], axis=1)
    masks = np.ascontiguousarray(masks).astype(BF)

    in_maps = []
    for c in range(cfg.NCORES):
        b, tp = divmod(c, cfg.TP)
        qsl = slice(tp * cfg.QCOLS, (tp + 1) * cfg.QCOLS)
        ksl = slice(tp * cfg.KCOLS, (tp + 1) * cfg.KCOLS)
        wq_l = wq[qsl].reshape(cfg.HL, HD, cfg.D)[:, perm, :].reshape(
            cfg.QCOLS, cfg.D)
        wk_l = wk[ksl].reshape(cfg.KVL, HD, cfg.D)[:, perm, :].reshape(
            cfg.KCOLS, cfg.D)
        wv_l = wv[ksl]
        wqkvT = np.ascontiguousarray(
            np.concatenate([wq_l, wk_l, wv_l], 0).T).astype(BF)
        osl = slice(tp * cfg.OUTC, (tp + 1) * cfg.OUTC)
        wo_l = wo[osl, :]                    # [OUTC, H*HD]
        m = {
            "xT": np.ascontiguousarray(x[b].T).astype(BF),
            "wqkvT": wqkvT,
            "p1": p1, "p2": p2, "masks": masks,
            "ones1": np.ones((128, 1), BF),
            "ones2": np.ones((1, 128), BF),
            "ident": np.eye(128, dtype=np.float32).astype(BF),
        }
        for p, (h0, h1) in enumerate(PASSES):
            cols = []
            for h in range(h0, h1):
                for r in range(cfg.TP):
                    g = r * cfg.HL + h
                    cols.append(wo_l[:, g * 128:(g + 1) * 128].T)
            m[f"woT{p}"] = np.ascontiguousarray(
                np.concatenate(cols, 0)).astype(BF)
        in_maps.append(m)
    return in_maps


def assemble(cfg, results):
    outs = []
    for b in range(cfg.B):
        parts = [results[b * cfg.TP + tp]["y"] for tp in range(cfg.TP)]
        outs.append(np.concatenate(parts, axis=1))
    return np.stack(outs, 0).astype(np.float32)


_CACHE = {}


def kernel(x, freq_cis, wq, wk, wv, wo):
    x = np.asarray(x, np.float32)
    freq_cis = np.asarray(freq_cis, np.float32)
    wq = np.asarray(wq, np.float32)
    wk = np.asarray(wk, np.float32)
    wv = np.asarray(wv, np.float32)
    wo = np.asarray(wo, np.float32)

    cfg = Cfg()
    if "nc" not in _CACHE:
        rg = [[g * cfg.TP + i for i in range(cfg.TP)] for g in range(cfg.DP)]
        _CACHE["nc"] = build_program(cfg, cfg.NCORES, rg)
    nc = _CACHE["nc"]
    in_maps = host_prep(cfg, x, freq_cis, wq, wk, wv, wo)
    res = run_bass_kernel_spmd(nc, in_maps, core_ids=list(range(cfg.NCORES)))
    return assemble(cfg, res.results)


if __name__ == "__main__":
    import reference
    inputs = {k: np.asarray(v) for k, v in reference.setup_inputs().items()}
    out = kernel(**inputs)
    exp = np.asarray(reference.reference(**inputs))
    err = np.abs(out - exp)
    denom = np.sqrt(np.mean(exp ** 2))
    print("max abs err:", err.max())
    print("rel err (rms):", np.sqrt(np.mean(err ** 2)) / denom)

